# revision 4
# baseline (speedup 1.0000x reference)
"""Trainium2 Bass kernel for nn_AttentionSimple (sparse_attention, 8 cores).

Algorithm: count-weighted vocab-space softmax — no per-token gathers.
Scores depend on s only through v = k[b, s], so group softmax terms by
vocabulary id:
    c[b, v]  = |{s : k[b, s] = v}|         (histogram of k, built on host)
    l[b, v]  = q[b] . embeddings[v]        (dense PE matmul, fp16)
    A        = c * exp(l)
    out[b]   = (sum_v A[b,v] * EW[v]) / (sum_v A[b,v])
    with EW  = embeddings @ W.T + b        (parameter prepacking, host)

Sharding: padded vocabulary (53248 = 416 chunks of 128) split across the
8 cores (52 chunks each); every core handles all 128 batch rows. Each
core returns partial numerators/denominators; the host unshard step sums
the 8 partials and divides (flash-style partial-softmax merge).

Per-core device pipeline (13 quads processed as 7 "octs" of 2 quads;
quad = 4 vocab chunks = 512 logit columns):
  - PE stage 1: 256-col fp16 matmuls (2 chunks stacked on the
    contraction dim via the dual-q trick), PSUM f32.
  - ACT: A1 = exp(ps) PSUM->SBUF bf16, one 1024-col pass per oct.
  - DVE: A2 = bf16(A1 * counts), one 1024-col pass per oct.
  - PE stage 2: per-chunk [128,3]x[128,128] bf16 matmuls, col-tiled
    via tile_position=(0, 32*(chunk%4)) so 4 run concurrently in
    distinct 32-column groups of the PE array; each group accumulates
    its chunks into acc[32j:32j+3, :] across the whole kernel.
  - Bulk DMAs split over two queues: embeddings on Sync, counts on
    GpSimd; tiny qe/st transfers lead on Sync/Scalar.
"""
import numpy as np

BATCH, SEQ, EMB, VOCAB, OUT = 128, 8192, 50, 50000, 2
N_CORES = 8
CSH = 52
NCHUNK = CSH * N_CORES
VPAD = NCHUNK * 128
VSH = CSH * 128
NPAIR = CSH // 2
NQUAD = NPAIR // 2
EPAD = 64
NQW = 2 * BATCH
OCT_QUADS = [2, 2, 2, 2, 2, 2, 1]

_CACHE = {}


def _build_nc():
    from contextlib import ExitStack

    import concourse.mybir as mybir
    import concourse.tile as tile
    from concourse import bacc

    f32 = mybir.dt.float32
    f16 = mybir.dt.float16
    bf16 = mybir.dt.bfloat16
    u8 = mybir.dt.uint8
    nc = bacc.Bacc("TRN2", target_bir_lowering=False, debug=False,
                   num_devices=N_CORES)

    qe_d = nc.dram_tensor("qe", [128, NQW + 512], f16, kind="ExternalInput")
    embT2_d = nc.dram_tensor("embT2", [128, NPAIR * 128 - 512], f16,
                             kind="ExternalInput")
    ct_d = nc.dram_tensor("ct", [128, NQUAD * 512], u8, kind="ExternalInput")
    st_d = nc.dram_tensor("st", [128, CSH * 3], bf16, kind="ExternalInput")
    o_d = nc.dram_tensor("o", [128, 128], f32, kind="ExternalOutput")

    f32r = mybir.dt.float32r
    with tile.TileContext(nc) as tc, ExitStack() as ctx:
        const_p = ctx.enter_context(tc.tile_pool(name="const", bufs=1))
        et_p = ctx.enter_context(tc.tile_pool(name="etp", bufs=6))
        ct_p = ctx.enter_context(tc.tile_pool(name="ctp", bufs=7))
        a1_p = ctx.enter_context(tc.tile_pool(name="a1p", bufs=3))
        a2_p = ctx.enter_context(tc.tile_pool(name="a2p", bufs=3))
        ps_p = ctx.enter_context(tc.tile_pool(name="ps", bufs=3, space="PSUM"))
        acc_p = ctx.enter_context(tc.tile_pool(name="acc", bufs=1,
                                               space="PSUM"))
        wps_p = ctx.enter_context(tc.tile_pool(name="wps", bufs=1,
                                               space="PSUM"))
        fin_p = ctx.enter_context(tc.tile_pool(name="fin", bufs=1))

        # warm-up matmuls: cover the ~5us cold-start latency of the first
        # DMA completion while ramping the PE HAM clock gate
        wtile = const_p.tile([128, 512], f32r)
        nc.vector.memset(wtile[:].bitcast(f32), 0.0)
        wps = wps_p.tile([128, 512], f32)
        for _ in range(6):
            nc.tensor.matmul(wps[:], lhsT=wtile[:, 0:128], rhs=wtile[:],
                             start=True, stop=True)

        qe_sb = const_p.tile([128, NQW + 512], f16)
        nc.sync.dma_start(qe_sb[:], qe_d.ap())
        st_sb = const_p.tile([128, CSH * 3], bf16)
        nc.scalar.dma_start(st_sb[:], st_d.ap())
        qw = qe_sb[:, 0:NQW]
        acc = acc_p.tile([128, 128], f32)

        # issue every bulk DMA upfront (per-transfer completion latency is
        # ~4.5us — it must pipeline, never sit on the critical path)
        ets, cts = [None], []
        quad0 = 0
        for oi, osz in enumerate(OCT_QUADS):
            if oi > 0:
                ett = et_p.tile([128, 2 * 256], f16, tag=f"et{oi}")
                nc.sync.dma_start(
                    ett[:, 0:osz * 256],
                    embT2_d.ap()[:, (quad0 - 2) * 256:
                                 (quad0 - 2 + osz) * 256])
                ets.append(ett)
            ctt = ct_p.tile([128, 2 * 512], u8, tag=f"ct{oi}")
            nc.gpsimd.dma_start(
                ctt[:, 0:osz * 512],
                ct_d.ap()[:, quad0 * 512:(quad0 + osz) * 512])
            cts.append(ctt)
            quad0 += osz

        quad0 = 0
        for oi, osz in enumerate(OCT_QUADS):
            et = qe_sb[:, NQW:] if oi == 0 else ets[oi][:]
            ctt = cts[oi]
            ps = ps_p.tile([128, 1024], f32)
            for lq in range(osz):
                for h in range(2):
                    nc.tensor.matmul(
                        ps[:, lq * 512 + h * 256:lq * 512 + (h + 1) * 256],
                        lhsT=et[:, lq * 256 + h * 128:
                                lq * 256 + h * 128 + 128],
                        rhs=qw,
                        start=True, stop=True,
                    )
            ncols = osz * 512
            a1 = a1_p.tile([128, 1024], bf16)
            nc.scalar.activation(a1[:, 0:ncols], ps[:, 0:ncols],
                                 mybir.ActivationFunctionType.Exp)
            a2 = a2_p.tile([128, 1024], bf16)
            nc.vector.tensor_mul(a2[:, 0:ncols], a1[:, 0:ncols],
                                 ctt[:, 0:ncols])
            if oi == len(OCT_QUADS) - 1:
                # bridge the last ACT->DVE drain so the HAM MID window
                # never sees an idle PE right before the epilogue
                for _ in range(2):
                    nc.tensor.matmul(wps[:], lhsT=wtile[:, 0:128],
                                     rhs=wtile[:], start=True, stop=True)
            for lc in range(4 * osz):
                c = 4 * quad0 + lc            # global chunk id
                j = c % 4                      # column group
                nc.tensor.matmul(
                    acc[32 * j:32 * j + 3, :],
                    lhsT=st_sb[:, c * 3:(c + 1) * 3],
                    rhs=a2[:, lc * 128:(lc + 1) * 128],
                    start=(c < 4), stop=(c >= 4 * NQUAD - 4),
                    tile_position=(0, 32 * j),
                    skip_group_check=True,
                )
            quad0 += osz

        osb = fin_p.tile([128, 128], f32)
        nc.vector.tensor_copy(osb[:], acc[:])
        nc.sync.dma_start(o_d.ap(), osb[:])
        # keep the Tensor sequencer warm through the output-DMA wait and
        # into walrus's semaphore-clear epilogue (cold NX clears at ~115ns
        # each vs ~57ns warm; Tensor clears 52 semaphores)
        for _ in range(10):
            nc.tensor.matmul(wps[:], lhsT=wtile[:, 0:128], rhs=wtile[:],
                             start=True, stop=True)

    nc.finalize()
    return nc


def _prep_inputs(q, k, embeddings, W, b):
    import ml_dtypes

    f16 = np.float16
    bf16 = ml_dtypes.bfloat16

    q = np.ascontiguousarray(q, dtype=np.float32)
    emb = np.ascontiguousarray(embeddings, dtype=np.float32)
    W = np.ascontiguousarray(W, dtype=np.float32)
    b = np.ascontiguousarray(b, dtype=np.float32)
    k = np.asarray(k)

    embT = np.zeros((EMB, VPAD), np.float32)
    embT[:, :VOCAB] = emb.T

    qw = np.zeros((128, NQW), np.float32)
    qw[:EMB, 0:BATCH] = q.T
    qw[EPAD:EPAD + EMB, BATCH:2 * BATCH] = q.T

    EWp = np.zeros((VPAD, OUT), np.float32)
    EWp[:VOCAB] = emb @ W.T + b[None, :]

    flat = (np.arange(BATCH, dtype=np.int64)[:, None] * VPAD
            + k.astype(np.int64)).ravel()
    C = np.bincount(flat, minlength=BATCH * VPAD).reshape(BATCH, VPAD)
    assert C.max() <= 255
    C = C.astype(np.uint8)

    in_maps = []
    for core in range(N_CORES):
        v0 = core * VSH
        blocks = embT[:, v0:v0 + VSH].reshape(EMB, CSH, 128)
        e2 = np.zeros((128, NPAIR, 128), np.float32)
        e2[:EMB] = blocks[:, 0::2, :]
        e2[EPAD:EPAD + EMB] = blocks[:, 1::2, :]
        e2 = e2.reshape(128, NPAIR * 128)
        qe = np.ascontiguousarray(np.concatenate(
            [qw, e2[:, 0:512]], axis=1)).astype(f16)
        embT2 = np.ascontiguousarray(e2[:, 512:]).astype(f16)

        # st[vrow, chunk, j]: j = (EW0, EW1, 1); chunk vocab = v0+c*128+vrow
        ew_blocks = EWp[v0:v0 + VSH].reshape(CSH, 128, OUT)
        st = np.zeros((128, CSH, 3), np.float32)
        st[:, :, 0:2] = ew_blocks.transpose(1, 0, 2)
        st[:, :, 2] = 1.0
        st = np.ascontiguousarray(st.reshape(128, CSH * 3)).astype(bf16)

        ct = np.ascontiguousarray(
            C[:, v0:v0 + VSH].reshape(BATCH, CSH, 128)
            .transpose(2, 1, 0).reshape(128, CSH * BATCH))
        in_maps.append({"embT2": embT2, "qe": qe, "st": st, "ct": ct})
    return in_maps


def _run_device(in_maps, **kwargs):
    from concourse.bass_utils import run_bass_kernel_spmd

    if "nc" not in _CACHE:
        _CACHE["nc"] = _build_nc()
    return run_bass_kernel_spmd(_CACHE["nc"], in_maps,
                                core_ids=list(range(N_CORES)), **kwargs)


def _unshard(res):
    P = np.zeros((128, 128), np.float64)
    for i in range(N_CORES):
        P += res.results[i]["o"].astype(np.float64)
    numer = np.zeros((OUT, BATCH), np.float64)
    denom = np.zeros(BATCH, np.float64)
    for j in range(4):
        numer += P[32 * j:32 * j + 2]
        denom += P[32 * j + 2]
    out = (numer / denom[None, :]).T
    return np.ascontiguousarray(out, dtype=np.float32)


def kernel(q, k, embeddings, W, b, **_unused):
    in_maps = _prep_inputs(q, k, embeddings, W, b)
    res = _run_device(in_maps)
    return _unshard(res)


# revision 8
# speedup vs baseline: 1.0664x; 1.0664x over previous
"""Trainium2 Bass kernel for nn_AttentionSimple (sparse_attention, 8 cores).

Algorithm: count-weighted vocab-space softmax — no per-token gathers.
Scores depend on s only through v = k[b, s], so group softmax terms by
vocabulary id:
    c[b, v]  = |{s : k[b, s] = v}|         (histogram of k, built on host)
    l[b, v]  = q[b] . embeddings[v]        (dense PE matmul, fp16)
    A        = c * exp(l)
    out[b]   = (sum_v A[b,v] * EW[v]) / (sum_v A[b,v])
    with EW  = embeddings @ W.T + b        (parameter prepacking, host)

Sharding: padded vocabulary (53248 = 416 chunks of 128) split across the
8 cores (52 chunks each); every core handles all 128 batch rows. Each
core returns partial numerators/denominators; the host unshard step sums
the 8 partials and divides (flash-style partial-softmax merge).

Per-core device pipeline (13 quads processed as 7 "octs" of 2 quads;
quad = 4 vocab chunks = 512 logit columns):
  - PE stage 1: 256-col fp16 matmuls (2 chunks stacked on the
    contraction dim via the dual-q trick), PSUM f32.
  - ACT: A1 = exp(ps) PSUM->SBUF bf16, one 1024-col pass per oct.
  - DVE: A2 = bf16(A1 * counts), one 1024-col pass per oct.
  - PE stage 2: per-chunk [128,3]x[128,128] bf16 matmuls, col-tiled
    via tile_position=(0, 32*(chunk%4)) so 4 run concurrently in
    distinct 32-column groups of the PE array; each group accumulates
    its chunks into acc[32j:32j+3, :] across the whole kernel.
  - Bulk DMAs split over two queues: embeddings on Sync, counts on
    GpSimd; tiny qe/st transfers lead on Sync/Scalar.
"""
import numpy as np

BATCH, SEQ, EMB, VOCAB, OUT = 128, 8192, 50, 50000, 2
N_CORES = 8
CSH = 52
NCHUNK = CSH * N_CORES
VPAD = NCHUNK * 128
VSH = CSH * 128
NPAIR = CSH // 2
NQUAD = NPAIR // 2
EPAD = 64
NQW = 2 * BATCH
OCT_QUADS = [2, 2, 2, 2, 2, 2, 1]

_CACHE = {}


def _build_nc():
    from contextlib import ExitStack

    import concourse.mybir as mybir
    import concourse.tile as tile
    from concourse import bacc

    f32 = mybir.dt.float32
    f16 = mybir.dt.float16
    bf16 = mybir.dt.bfloat16
    u8 = mybir.dt.uint8
    nc = bacc.Bacc("TRN2", target_bir_lowering=False, debug=False,
                   num_devices=N_CORES)

    qe_d = nc.dram_tensor("qe", [128, NQW + 512], f16, kind="ExternalInput")
    embT2_d = nc.dram_tensor("embT2", [128, NPAIR * 128 - 512], f16,
                             kind="ExternalInput")
    ct_d = nc.dram_tensor("ct", [128, NQUAD * 512], u8, kind="ExternalInput")
    st_d = nc.dram_tensor("st", [128, CSH * 3], bf16, kind="ExternalInput")
    o_d = nc.dram_tensor("o", [128, 128], f32, kind="ExternalOutput")

    f32r = mybir.dt.float32r
    with tile.TileContext(nc) as tc, ExitStack() as ctx:
        const_p = ctx.enter_context(tc.tile_pool(name="const", bufs=1))
        et_p = ctx.enter_context(tc.tile_pool(name="etp", bufs=6))
        ct_p = ctx.enter_context(tc.tile_pool(name="ctp", bufs=7))
        a1_p = ctx.enter_context(tc.tile_pool(name="a1p", bufs=3))
        a2_p = ctx.enter_context(tc.tile_pool(name="a2p", bufs=3))
        ps_p = ctx.enter_context(tc.tile_pool(name="ps", bufs=3, space="PSUM"))
        acc_p = ctx.enter_context(tc.tile_pool(name="acc", bufs=1,
                                               space="PSUM"))
        wps_p = ctx.enter_context(tc.tile_pool(name="wps", bufs=1,
                                               space="PSUM"))
        fin_p = ctx.enter_context(tc.tile_pool(name="fin", bufs=1))

        # warm-up matmuls: cover the ~5us cold-start latency of the first
        # DMA completion while ramping the PE HAM clock gate
        wtile = const_p.tile([128, 512], f32r)
        nc.vector.memset(wtile[:].bitcast(f32), 0.0)
        wps = wps_p.tile([128, 512], f32)
        for _ in range(6):
            nc.tensor.matmul(wps[:], lhsT=wtile[:, 0:128], rhs=wtile[:],
                             start=True, stop=True)

        qe_sb = const_p.tile([128, NQW + 512], f16)
        nc.sync.dma_start(qe_sb[:], qe_d.ap())
        st_sb = const_p.tile([128, CSH * 3], bf16)
        nc.scalar.dma_start(st_sb[:], st_d.ap())
        qw = qe_sb[:, 0:NQW]
        acc = acc_p.tile([128, 128], f32)

        # issue every bulk DMA upfront (per-transfer completion latency is
        # ~4.5us — it must pipeline, never sit on the critical path)
        ets, cts = [None], []
        quad0 = 0
        for oi, osz in enumerate(OCT_QUADS):
            if oi > 0:
                ett = et_p.tile([128, 2 * 256], f16, tag=f"et{oi}")
                nc.sync.dma_start(
                    ett[:, 0:osz * 256],
                    embT2_d.ap()[:, (quad0 - 2) * 256:
                                 (quad0 - 2 + osz) * 256])
                ets.append(ett)
            ctt = ct_p.tile([128, 2 * 512], u8, tag=f"ct{oi}")
            nc.gpsimd.dma_start(
                ctt[:, 0:osz * 512],
                ct_d.ap()[:, quad0 * 512:(quad0 + osz) * 512])
            cts.append(ctt)
            quad0 += osz

        quad0 = 0
        for oi, osz in enumerate(OCT_QUADS):
            et = qe_sb[:, NQW:] if oi == 0 else ets[oi][:]
            ctt = cts[oi]
            ps = ps_p.tile([128, 1024], f32)
            for lq in range(osz):
                for h in range(2):
                    nc.tensor.matmul(
                        ps[:, lq * 512 + h * 256:lq * 512 + (h + 1) * 256],
                        lhsT=et[:, lq * 256 + h * 128:
                                lq * 256 + h * 128 + 128],
                        rhs=qw,
                        start=True, stop=True,
                    )
            ncols = osz * 512
            a1 = a1_p.tile([128, 1024], bf16)
            nc.scalar.activation(a1[:, 0:ncols], ps[:, 0:ncols],
                                 mybir.ActivationFunctionType.Exp)
            a2 = a2_p.tile([128, 1024], bf16)
            nc.vector.tensor_mul(a2[:, 0:ncols], a1[:, 0:ncols],
                                 ctt[:, 0:ncols])
            for lc in range(4 * osz):
                c = 4 * quad0 + lc            # global chunk id
                j = c % 4                      # column group
                nc.tensor.matmul(
                    acc[32 * j:32 * j + 3, :],
                    lhsT=st_sb[:, c * 3:(c + 1) * 3],
                    rhs=a2[:, lc * 128:(lc + 1) * 128],
                    start=(c < 4), stop=(c >= 4 * NQUAD - 4),
                    tile_position=(0, 32 * j),
                    skip_group_check=True,
                )
            quad0 += osz

        # Sync-queue keepalive: a tiny transfer whose buffer (an a2-pool
        # rotation) only frees after oct-4's stage-2 matmuls, so it issues
        # late and keeps the DMA engines warm for the output transfer
        ka = a2_p.tile([128, 1024], bf16, tag="ct")
        nc.sync.dma_start(ka[:, 0:32].bitcast(f16), qe_d.ap()[:, 0:32])

        osb = fin_p.tile([128, 128], f32)
        nc.vector.tensor_copy(osb[:], acc[:])
        nc.sync.dma_start(o_d.ap(), osb[:])
        # keep the Tensor sequencer warm through the output-DMA wait and
        # into walrus's semaphore-clear epilogue (cold NX clears at ~154ns
        # each vs ~60ns warm; Tensor clears 53 semaphores). lhsT reads osb
        # so the scheduler cannot hoist these before the final copy.
        osb_b = osb[:].bitcast(bf16)
        wtile_b = wtile[:].bitcast(bf16)
        for _ in range(14):
            nc.tensor.matmul(wps[:], lhsT=osb_b[:, 0:128],
                             rhs=wtile_b[:, 0:512],
                             start=True, stop=True)

    nc.finalize()
    return nc


def _prep_inputs(q, k, embeddings, W, b):
    import ml_dtypes

    f16 = np.float16
    bf16 = ml_dtypes.bfloat16

    q = np.ascontiguousarray(q, dtype=np.float32)
    emb = np.ascontiguousarray(embeddings, dtype=np.float32)
    W = np.ascontiguousarray(W, dtype=np.float32)
    b = np.ascontiguousarray(b, dtype=np.float32)
    k = np.asarray(k)

    embT = np.zeros((EMB, VPAD), np.float32)
    embT[:, :VOCAB] = emb.T

    qw = np.zeros((128, NQW), np.float32)
    qw[:EMB, 0:BATCH] = q.T
    qw[EPAD:EPAD + EMB, BATCH:2 * BATCH] = q.T

    EWp = np.zeros((VPAD, OUT), np.float32)
    EWp[:VOCAB] = emb @ W.T + b[None, :]

    flat = (np.arange(BATCH, dtype=np.int64)[:, None] * VPAD
            + k.astype(np.int64)).ravel()
    C = np.bincount(flat, minlength=BATCH * VPAD).reshape(BATCH, VPAD)
    assert C.max() <= 255
    C = C.astype(np.uint8)

    in_maps = []
    for core in range(N_CORES):
        v0 = core * VSH
        blocks = embT[:, v0:v0 + VSH].reshape(EMB, CSH, 128)
        e2 = np.zeros((128, NPAIR, 128), np.float32)
        e2[:EMB] = blocks[:, 0::2, :]
        e2[EPAD:EPAD + EMB] = blocks[:, 1::2, :]
        e2 = e2.reshape(128, NPAIR * 128)
        qe = np.ascontiguousarray(np.concatenate(
            [qw, e2[:, 0:512]], axis=1)).astype(f16)
        embT2 = np.ascontiguousarray(e2[:, 512:]).astype(f16)

        # st[vrow, chunk, j]: j = (EW0, EW1, 1); chunk vocab = v0+c*128+vrow
        ew_blocks = EWp[v0:v0 + VSH].reshape(CSH, 128, OUT)
        st = np.zeros((128, CSH, 3), np.float32)
        st[:, :, 0:2] = ew_blocks.transpose(1, 0, 2)
        st[:, :, 2] = 1.0
        st = np.ascontiguousarray(st.reshape(128, CSH * 3)).astype(bf16)

        ct = np.ascontiguousarray(
            C[:, v0:v0 + VSH].reshape(BATCH, CSH, 128)
            .transpose(2, 1, 0).reshape(128, CSH * BATCH))
        in_maps.append({"embT2": embT2, "qe": qe, "st": st, "ct": ct})
    return in_maps


def _run_device(in_maps, **kwargs):
    from concourse.bass_utils import run_bass_kernel_spmd

    if "nc" not in _CACHE:
        _CACHE["nc"] = _build_nc()
    return run_bass_kernel_spmd(_CACHE["nc"], in_maps,
                                core_ids=list(range(N_CORES)), **kwargs)


def _unshard(res):
    P = np.zeros((128, 128), np.float64)
    for i in range(N_CORES):
        P += res.results[i]["o"].astype(np.float64)
    numer = np.zeros((OUT, BATCH), np.float64)
    denom = np.zeros(BATCH, np.float64)
    for j in range(4):
        numer += P[32 * j:32 * j + 2]
        denom += P[32 * j + 2]
    out = (numer / denom[None, :]).T
    return np.ascontiguousarray(out, dtype=np.float32)


def kernel(q, k, embeddings, W, b, **_unused):
    in_maps = _prep_inputs(q, k, embeddings, W, b)
    res = _run_device(in_maps)
    return _unshard(res)


# revision 9
# speedup vs baseline: 1.1737x; 1.1006x over previous
"""Trainium2 Bass kernel for nn_AttentionSimple (sparse_attention, 8 cores).

Algorithm: count-weighted vocab-space softmax — no per-token gathers.
Scores depend on s only through v = k[b, s], so group softmax terms by
vocabulary id:
    c[b, v]  = |{s : k[b, s] = v}|         (histogram of k, built on host)
    l[b, v]  = q[b] . embeddings[v]        (dense PE matmul, fp16)
    A        = c * exp(l)
    out[b]   = (sum_v A[b,v] * EW[v]) / (sum_v A[b,v])
    with EW  = embeddings @ W.T + b        (parameter prepacking, host)

Sharding: padded vocabulary (53248 = 416 chunks of 128) split across the
8 cores (52 chunks each); every core handles all 128 batch rows. Each
core returns partial numerators/denominators; the host unshard step sums
the 8 partials and divides (flash-style partial-softmax merge).

Per-core device pipeline (13 quads processed as 7 "octs" of 2 quads;
quad = 4 vocab chunks = 512 logit columns):
  - PE stage 1: 256-col fp16 matmuls (2 chunks stacked on the
    contraction dim via the dual-q trick), PSUM f32.
  - ACT: A1 = exp(ps) PSUM->SBUF bf16, one 1024-col pass per oct.
  - DVE: A2 = bf16(A1 * counts), one 1024-col pass per oct.
  - PE stage 2: per-chunk [128,3]x[128,128] bf16 matmuls, col-tiled
    via tile_position=(0, 32*(chunk%4)) so 4 run concurrently in
    distinct 32-column groups of the PE array; each group accumulates
    its chunks into acc[32j:32j+3, :] across the whole kernel.
  - Bulk DMAs split over two queues: embeddings on Sync, counts on
    GpSimd; tiny qe/st transfers lead on Sync/Scalar.
"""
import numpy as np

BATCH, SEQ, EMB, VOCAB, OUT = 128, 8192, 50, 50000, 2
N_CORES = 8
CSH = 52
NCHUNK = CSH * N_CORES
VPAD = NCHUNK * 128
VSH = CSH * 128
NPAIR = CSH // 2
NQUAD = NPAIR // 2
EPAD = 64
NQW = 2 * BATCH
OCT_QUADS = [2, 2, 2, 2, 2, 2, 1]

_CACHE = {}


def _build_nc():
    from contextlib import ExitStack

    import concourse.mybir as mybir
    import concourse.tile as tile
    from concourse import bacc

    f32 = mybir.dt.float32
    f16 = mybir.dt.float16
    bf16 = mybir.dt.bfloat16
    u8 = mybir.dt.uint8
    nc = bacc.Bacc("TRN2", target_bir_lowering=False, debug=False,
                   num_devices=N_CORES)

    qe_d = nc.dram_tensor("qe", [128, NQW + 512], f16, kind="ExternalInput")
    embT2_d = nc.dram_tensor("embT2", [128, NPAIR * 128 - 512], f16,
                             kind="ExternalInput")
    ct_d = nc.dram_tensor("ct", [128, NQUAD * 512], u8, kind="ExternalInput")
    st_d = nc.dram_tensor("st", [128, CSH * 3], bf16, kind="ExternalInput")
    o_d = nc.dram_tensor("o", [128, 128], f32, kind="ExternalOutput")

    f32r = mybir.dt.float32r
    with tile.TileContext(nc) as tc, ExitStack() as ctx:
        const_p = ctx.enter_context(tc.tile_pool(name="const", bufs=1))
        et_p = ctx.enter_context(tc.tile_pool(name="etp", bufs=6))
        ct_p = ctx.enter_context(tc.tile_pool(name="ctp", bufs=7))
        a1_p = ctx.enter_context(tc.tile_pool(name="a1p", bufs=3))
        a2_p = ctx.enter_context(tc.tile_pool(name="a2p", bufs=3))
        ps_p = ctx.enter_context(tc.tile_pool(name="ps", bufs=3, space="PSUM"))
        acc_p = ctx.enter_context(tc.tile_pool(name="acc", bufs=1,
                                               space="PSUM"))
        wps_p = ctx.enter_context(tc.tile_pool(name="wps", bufs=1,
                                               space="PSUM"))
        fin_p = ctx.enter_context(tc.tile_pool(name="fin", bufs=1))

        # warm-up matmuls: cover the ~5us cold-start latency of the first
        # DMA completion while ramping the PE HAM clock gate
        wtile = const_p.tile([128, 512], f32r)
        nc.vector.memset(wtile[:].bitcast(f32), 0.0)
        wps = wps_p.tile([128, 512], f32)
        for _ in range(6):
            nc.tensor.matmul(wps[:], lhsT=wtile[:, 0:128], rhs=wtile[:],
                             start=True, stop=True)

        qe_sb = const_p.tile([128, NQW + 512], f16)
        nc.sync.dma_start(qe_sb[:], qe_d.ap())
        st_sb = const_p.tile([128, CSH * 3], bf16)
        nc.scalar.dma_start(st_sb[:], st_d.ap())
        qw = qe_sb[:, 0:NQW]
        acc = acc_p.tile([128, 128], f32)

        # issue every bulk DMA upfront (per-transfer completion latency is
        # ~4.5us — it must pipeline, never sit on the critical path)
        ets, cts = [None], []
        quad0 = 0
        for oi, osz in enumerate(OCT_QUADS):
            if oi > 0:
                ett = et_p.tile([128, 2 * 256], f16, tag=f"et{oi}")
                nc.sync.dma_start(
                    ett[:, 0:osz * 256],
                    embT2_d.ap()[:, (quad0 - 2) * 256:
                                 (quad0 - 2 + osz) * 256])
                ets.append(ett)
            ctt = ct_p.tile([128, 2 * 512], u8, tag=f"ct{oi}")
            nc.gpsimd.dma_start(
                ctt[:, 0:osz * 512],
                ct_d.ap()[:, quad0 * 512:(quad0 + osz) * 512])
            cts.append(ctt)
            quad0 += osz

        quad0 = 0
        for oi, osz in enumerate(OCT_QUADS):
            et = qe_sb[:, NQW:] if oi == 0 else ets[oi][:]
            ctt = cts[oi]
            ps = ps_p.tile([128, 1024], f32)
            for lq in range(osz):
                for h in range(2):
                    nc.tensor.matmul(
                        ps[:, lq * 512 + h * 256:lq * 512 + (h + 1) * 256],
                        lhsT=et[:, lq * 256 + h * 128:
                                lq * 256 + h * 128 + 128],
                        rhs=qw,
                        start=True, stop=True,
                    )
            ncols = osz * 512
            a1 = a1_p.tile([128, 1024], bf16)
            nc.scalar.activation(a1[:, 0:ncols], ps[:, 0:ncols],
                                 mybir.ActivationFunctionType.Exp)
            a2 = a2_p.tile([128, 1024], bf16)
            nc.vector.tensor_mul(a2[:, 0:ncols], a1[:, 0:ncols],
                                 ctt[:, 0:ncols])
            for lc in range(4 * osz):
                c = 4 * quad0 + lc            # global chunk id
                j = c % 4                      # column group
                nc.tensor.matmul(
                    acc[32 * j:32 * j + 3, :],
                    lhsT=st_sb[:, c * 3:(c + 1) * 3],
                    rhs=a2[:, lc * 128:(lc + 1) * 128],
                    start=(c < 4), stop=(c >= 4 * NQUAD - 4),
                    tile_position=(0, 32 * j),
                    skip_group_check=True,
                )
            quad0 += osz

        osb = fin_p.tile([128, 128], f32)
        nc.vector.tensor_copy(osb[:], acc[:])
        nc.sync.dma_start(o_d.ap(), osb[:])

    nc.finalize()
    return nc


def _prep_inputs(q, k, embeddings, W, b):
    import ml_dtypes

    f16 = np.float16
    bf16 = ml_dtypes.bfloat16

    q = np.ascontiguousarray(q, dtype=np.float32)
    emb = np.ascontiguousarray(embeddings, dtype=np.float32)
    W = np.ascontiguousarray(W, dtype=np.float32)
    b = np.ascontiguousarray(b, dtype=np.float32)
    k = np.asarray(k)

    embT = np.zeros((EMB, VPAD), np.float32)
    embT[:, :VOCAB] = emb.T

    qw = np.zeros((128, NQW), np.float32)
    qw[:EMB, 0:BATCH] = q.T
    qw[EPAD:EPAD + EMB, BATCH:2 * BATCH] = q.T

    EWp = np.zeros((VPAD, OUT), np.float32)
    EWp[:VOCAB] = emb @ W.T + b[None, :]

    flat = (np.arange(BATCH, dtype=np.int64)[:, None] * VPAD
            + k.astype(np.int64)).ravel()
    C = np.bincount(flat, minlength=BATCH * VPAD).reshape(BATCH, VPAD)
    assert C.max() <= 255
    C = C.astype(np.uint8)

    in_maps = []
    for core in range(N_CORES):
        v0 = core * VSH
        blocks = embT[:, v0:v0 + VSH].reshape(EMB, CSH, 128)
        e2 = np.zeros((128, NPAIR, 128), np.float32)
        e2[:EMB] = blocks[:, 0::2, :]
        e2[EPAD:EPAD + EMB] = blocks[:, 1::2, :]
        e2 = e2.reshape(128, NPAIR * 128)
        qe = np.ascontiguousarray(np.concatenate(
            [qw, e2[:, 0:512]], axis=1)).astype(f16)
        embT2 = np.ascontiguousarray(e2[:, 512:]).astype(f16)

        # st[vrow, chunk, j]: j = (EW0, EW1, 1); chunk vocab = v0+c*128+vrow
        ew_blocks = EWp[v0:v0 + VSH].reshape(CSH, 128, OUT)
        st = np.zeros((128, CSH, 3), np.float32)
        st[:, :, 0:2] = ew_blocks.transpose(1, 0, 2)
        st[:, :, 2] = 1.0
        st = np.ascontiguousarray(st.reshape(128, CSH * 3)).astype(bf16)

        ct = np.ascontiguousarray(
            C[:, v0:v0 + VSH].reshape(BATCH, CSH, 128)
            .transpose(2, 1, 0).reshape(128, CSH * BATCH))
        in_maps.append({"embT2": embT2, "qe": qe, "st": st, "ct": ct})
    return in_maps


def _run_device(in_maps, **kwargs):
    from concourse.bass_utils import run_bass_kernel_spmd

    if "nc" not in _CACHE:
        _CACHE["nc"] = _build_nc()
    return run_bass_kernel_spmd(_CACHE["nc"], in_maps,
                                core_ids=list(range(N_CORES)), **kwargs)


def _unshard(res):
    P = np.zeros((128, 128), np.float64)
    for i in range(N_CORES):
        P += res.results[i]["o"].astype(np.float64)
    numer = np.zeros((OUT, BATCH), np.float64)
    denom = np.zeros(BATCH, np.float64)
    for j in range(4):
        numer += P[32 * j:32 * j + 2]
        denom += P[32 * j + 2]
    out = (numer / denom[None, :]).T
    return np.ascontiguousarray(out, dtype=np.float32)


def kernel(q, k, embeddings, W, b, **_unused):
    in_maps = _prep_inputs(q, k, embeddings, W, b)
    res = _run_device(in_maps)
    return _unshard(res)
